# revision 24
# baseline (speedup 1.0000x reference)
"""AdaMoLE (LoRA-MoE routing) Trainium2 kernel, data-parallel over tokens on 8 cores.

Math (per token n):
    logits = x @ Wr.T + br                 [E]
    gate   = softmax(logits)
    thr    = sigmoid(x @ Wt.T + bt)        [1]
    w      = relu(8*gate - thr); w /= max(sum(w), eps)   (scale-invariant vs ref)
    h      = x @ A_all                     [E*R]
    out    = (h * rep(w)) @ (B_all * SCALING)

Layout: x is STATIONARY in mm1 — one fused pass per 128-d chunk computes both
h and the 9 routing logits into one [128tok, 137] psum tile (the baseline
streamed x through PE twice).  Routing math runs in token-partition layout on
ACT/DVE with free-dim reductions; the routing weights are applied with a
single stride-0 broadcast multiply.  hw is PE-transposed to [er, tok] for mm2.
I/O is bf16 both ways (host casts), halving DMA volume vs f32.
"""

import sys

sys.path.insert(0, "/opt/trn_rl_repo")

import numpy as np
import ml_dtypes

import concourse.bacc as bacc
import concourse.mybir as mybir
import concourse.tile as tile
from concourse.bass_utils import run_bass_kernel_spmd
from contextlib import ExitStack

F32 = mybir.dt.float32
BF16 = mybir.dt.bfloat16
AF = mybir.ActivationFunctionType

B, S, D, DOUT = 4, 4096, 4096, 4096
R, E, SCALING = 16, 8, 8.0 / 16
NCORES = 8
N = B * S
NTOK = N // NCORES        # 2048 tokens per core
BS = 128                  # tokens per block (= stationary width)
NBLK = NTOK // BS         # 16
NDC = D // 128            # 32 contraction chunks
ER = E * R                # 128
CW = ER + 9               # fused mm1 output width (h ++ router logits)
NOB = DOUT // 512         # 8 output column blocks

_CACHE = {}


XGRP = 4                  # blocks per X DMA (4 MB transfers, 32 KB lines)
OGRP = 4                  # blocks per OUT DMA (4 MB transfers, 32 KB lines)


def _build(reps=1, loop=False, do_compute=True, do_dma=True,
           x_rings=("sync",), o_rings=("scalar",), xchunk=1):
    nc = bacc.Bacc("TRN2", debug=False, num_devices=NCORES)

    # X and OUT are partition-contiguous so DMA lines are 32KB/16KB per
    # partition (1MB-per-block layouts cap at ~78% of HBM bandwidth)
    X = nc.declare_dram_parameter("X", [128, NBLK * NDC * BS], BF16, isOutput=False)
    AWc = nc.declare_dram_parameter("AWc", [128, NDC * CW], BF16, isOutput=False)
    BIASR = nc.declare_dram_parameter("BIASR", [1, CW], BF16, isOutput=False)
    IDN = nc.declare_dram_parameter("IDN", [128, 128], BF16, isOutput=False)
    Bl = nc.declare_dram_parameter("Bl", [ER, DOUT], BF16, isOutput=False)
    OUT = nc.declare_dram_parameter("out", [128, NBLK * DOUT], BF16, isOutput=True)

    with tile.TileContext(nc) as tc, ExitStack() as ctx:
        wpool = ctx.enter_context(tc.tile_pool(name="w", bufs=1))
        xpool = ctx.enter_context(tc.tile_pool(name="x", bufs=3))
        opool = ctx.enter_context(tc.tile_pool(name="o", bufs=2))
        spool = ctx.enter_context(tc.tile_pool(name="s", bufs=2))
        hwpool = ctx.enter_context(tc.tile_pool(name="hw", bufs=2))
        ph = ctx.enter_context(tc.tile_pool(name="ph", bufs=3, space="PSUM"))
        pt = ctx.enter_context(tc.tile_pool(name="pt", bufs=2, space="PSUM"))
        po = ctx.enter_context(tc.tile_pool(name="po", bufs=3, space="PSUM"))

        # weights on the scalar ring (stores ring, idle at start) so X can
        # start streaming on the sync ring immediately
        AWc_sb = wpool.tile([128, NDC * CW], BF16, tag="AWc")
        nc.scalar.dma_start(out=AWc_sb[:], in_=AWc[:])
        biasr_sb = wpool.tile([1, CW], BF16, tag="biasr")
        nc.scalar.dma_start(out=biasr_sb[:], in_=BIASR[:])
        B_sb = wpool.tile([ER, DOUT], BF16, tag="B")
        nc.scalar.dma_start(out=B_sb[:], in_=Bl[:])
        idn_sb = wpool.tile([128, 128], BF16, tag="IDN")
        nc.scalar.dma_start(out=idn_sb[:], in_=IDN[:])
        ones1 = wpool.tile([1, 128], BF16, tag="ones1")
        nc.vector.memset(ones1[:], 1.0)

        def eng(name):
            return getattr(nc, name)

        BLKC = NDC * BS  # 4096 cols per block

        def emit_all():
            xg = []
            for g in range(NBLK // XGRP):
                xt = xpool.tile([128, XGRP * BLKC], BF16, tag="xg")
                if do_dma:
                    ring = x_rings[g % len(x_rings)]
                    eng(ring).dma_start(
                        out=xt[:], in_=X[:, g * XGRP * BLKC : (g + 1) * XGRP * BLKC]
                    )
                elif g == 0:
                    nc.vector.memset(xt[:], 0.01)
                xg.append(xt)
            if not do_dma:
                xg = [xg[0]] * len(xg)

            def xslice(b, c0, c1):
                t = xg[b // XGRP]
                off = (b % XGRP) * BLKC
                return t[:, off + c0 : off + c1]

            ps_l = [None] * NBLK
            hw_l = [None] * NBLK
            hwT_l = [None] * NBLK

            def stA(b):  # mm1: fused h + routing logits, x stationary
                ps = ph.tile([128, CW], F32, tag="ps")
                for dc in range(NDC):
                    nc.tensor.matmul(
                        ps[:],
                        xslice(b, dc * BS, (dc + 1) * BS),
                        AWc_sb[:, dc * CW : (dc + 1) * CW],
                        start=(dc == 0),
                        stop=False,
                    )
                nc.tensor.matmul(ps[:], ones1[:], biasr_sb[:], start=False, stop=True)
                ps_l[b] = ps

            def stB(b):  # routing math + broadcast-weight multiply
                ps = ps_l[b]
                eexp = spool.tile([128, E], F32, tag="eexp")
                S1 = spool.tile([128, 1], F32, tag="S1")
                nc.scalar.activation(eexp[:], ps[:, ER : ER + E], AF.Exp, accum_out=S1[:])
                # sigmoid via exp so ACT stays on one act-func set (no
                # 1.3us LUT reloads): thr/E = 1 / (E * (1 + exp(-z)))
                texp = spool.tile([128, 1], F32, tag="texp")
                nc.scalar.activation(texp[:], ps[:, ER + E : ER + E + 1], AF.Exp, scale=-1.0)
                u = spool.tile([128, 1], F32, tag="u")
                nc.vector.tensor_scalar(
                    u[:], texp[:], 1.0, float(E),
                    mybir.AluOpType.add, mybir.AluOpType.mult,
                )
                thr8 = spool.tile([128, 1], F32, tag="thr8")
                nc.vector.reciprocal(thr8[:], u[:])
                sg1 = spool.tile([128, 1], F32, tag="sg1")
                nc.vector.reciprocal(sg1[:], S1[:])
                adapted = spool.tile([128, E], F32, tag="ad")
                nc.vector.tensor_scalar(
                    adapted[:], eexp[:], sg1[:], thr8[:],
                    mybir.AluOpType.mult, mybir.AluOpType.subtract,
                )
                wrel = spool.tile([128, E], F32, tag="wr")
                S2 = spool.tile([128, 1], F32, tag="S2")
                nc.vector.tensor_scalar(
                    wrel[:], adapted[:], 0.0, 0.0,
                    mybir.AluOpType.max, mybir.AluOpType.add, accum_out=S2[:],
                )
                S2g = spool.tile([128, 1], F32, tag="S2g")
                nc.vector.tensor_scalar_max(S2g[:], S2[:], 1e-30)
                sr = spool.tile([128, 1], F32, tag="sr")
                nc.vector.reciprocal(sr[:], S2g[:])
                wfin = spool.tile([128, E], F32, tag="wf")
                nc.vector.tensor_scalar_mul(wfin[:], wrel[:], sr[:])
                hw_t = hwpool.tile([128, ER], BF16, tag="hwt")
                nc.vector.tensor_mul(
                    hw_t[:].rearrange("p (e r) -> p e r", e=E),
                    ps[:, 0:ER].rearrange("p (e r) -> p e r", e=E),
                    wfin[:].unsqueeze(2).broadcast_to([128, E, R]),
                )
                hw_l[b] = hw_t

            def stC(b):  # PE transpose [tok, er] -> [er, tok]
                tp = pt.tile([128, 128], BF16, tag="tp")
                nc.tensor.transpose(tp[:], hw_l[b][:], idn_sb[:])
                hwT_l[b] = tp

            def stD(b):  # psum -> sbuf for mm2 stationary
                hs = hwpool.tile([128, 128], BF16, tag="hwT")
                nc.vector.tensor_copy(hs[:], hwT_l[b][:])
                hwT_l[b] = hs

            o_cur = [None]

            def stEFG(b):  # mm2 + staged copies + grouped store
                if b % OGRP == 0:
                    o_new = opool.tile([128, OGRP * DOUT], BF16, tag="osb")
                    o_cur[0] = o_new
                o_sb = o_cur[0]
                ob = (b % OGRP) * DOUT
                for nb in range(NOB):
                    o_ps = po.tile([128, 512], F32, tag="o")
                    nc.tensor.matmul(
                        o_ps[:],
                        hwT_l[b][:],
                        B_sb[:, nb * 512 : (nb + 1) * 512],
                        start=True,
                        stop=True,
                    )
                    if nb % 8 < 5:
                        nc.scalar.activation(
                            o_sb[:, ob + nb * 512 : ob + (nb + 1) * 512], o_ps[:], AF.Copy
                        )
                    else:
                        nc.vector.tensor_copy(
                            o_sb[:, ob + nb * 512 : ob + (nb + 1) * 512], o_ps[:]
                        )
                if do_dma and b % OGRP == OGRP - 1:
                    g = b // OGRP
                    ring = o_rings[g % len(o_rings)]
                    eng(ring).dma_start(
                        out=OUT[:, g * OGRP * DOUT : (g + 1) * OGRP * DOUT],
                        in_=o_sb[:],
                    )

            if not do_compute:
                # DMA-only ablation: consume each X group with a 1-col matmul
                # (so loads stay on the critical path), store a constant o_sb
                o_sb = wpool.tile([128, OGRP * DOUT], BF16, tag="osbc")
                nc.gpsimd.memset(o_sb[:], 0.02)
                for g in range(NBLK // XGRP):
                    acc = po.tile([128, 1], F32, tag="acc")
                    nc.tensor.matmul(
                        acc[:], xg[g][:, 0:128], xg[g][:, 0:1], start=True, stop=True
                    )
                for g in range(NBLK // OGRP):
                    if do_dma:
                        ring = o_rings[g % len(o_rings)]
                        eng(ring).dma_start(
                            out=OUT[:, g * OGRP * DOUT : (g + 1) * OGRP * DOUT],
                            in_=o_sb[:],
                        )
                return

            # 2-deep software pipeline so PE never waits on the ACT/DVE
            # routing chain or the hwT copy
            stA(0); stB(0)
            stA(1); stB(1)
            stC(0); stD(0)
            for b in range(NBLK):
                if b + 2 < NBLK:
                    stA(b + 2); stB(b + 2)
                stEFG(b)
                if b + 1 < NBLK:
                    stC(b + 1); stD(b + 1)

        if loop:
            with tc.For_i(0, reps, 1):
                emit_all()
        else:
            for _ in range(reps):
                emit_all()

    nc.compile()
    return nc


def _prep_consts(Wr, br, Wt, bt, A, Bw):
    bf = ml_dtypes.bfloat16
    A_all = np.asarray(A, np.float32).transpose(1, 0, 2).reshape(D, ER)  # [d, er]
    Wcat = np.concatenate(
        [np.asarray(Wr, np.float32).T, np.asarray(Wt, np.float32).T], axis=1
    )  # [d, 9]
    AWc_h = np.concatenate(
        [A_all.reshape(NDC, 128, ER), Wcat.reshape(NDC, 128, 9)], axis=2
    )  # [NDC, 128, CW]
    AWc_host = np.ascontiguousarray(
        AWc_h.transpose(1, 0, 2).reshape(128, NDC * CW)
    ).astype(bf)
    biasr = np.zeros((1, CW), np.float32)
    biasr[0, ER : ER + E] = np.asarray(br, np.float32)
    biasr[0, ER + E] = np.float32(np.asarray(bt).reshape(()))
    B_host = (np.asarray(Bw, np.float32).reshape(ER, DOUT) * SCALING).astype(bf)
    idn = np.eye(128, dtype=np.float32).astype(bf)
    return {
        "AWc": AWc_host,
        "BIASR": biasr.astype(bf),
        "IDN": idn,
        "Bl": B_host,
    }


def _prep_x(xs):
    """Per-core shard [NTOK, D] -> [128, NBLK*NDC*BS] bf16,
    partition-contiguous: partition p holds [blk, dc, t] so each X DMA reads
    XGRP*8KB contiguous per partition."""
    arr = (
        np.asarray(xs, np.float32)
        .reshape(NBLK, BS, NDC, 128)
        .transpose(3, 0, 2, 1)  # [p, blk, dc, t]
        .reshape(128, NBLK * NDC * BS)
    )
    return np.ascontiguousarray(arr).astype(ml_dtypes.bfloat16)


def kernel(x, Wr, br, Wt, bt, A, Bw, _trace=False, _trace_kwargs=None):
    if "nc" not in _CACHE:
        _CACHE["nc"] = _build()
    nc = _CACHE["nc"]

    consts = _prep_consts(Wr, br, Wt, bt, A, Bw)
    xf = np.asarray(x, np.float32).reshape(N, D)
    in_maps = []
    for c in range(NCORES):
        Xh = _prep_x(xf[c * NTOK : (c + 1) * NTOK])
        in_maps.append({"X": Xh, **consts})

    res = run_bass_kernel_spmd(
        nc,
        in_maps,
        core_ids=list(range(NCORES)),
        trace=_trace,
        **(_trace_kwargs or {}),
    )
    # OUT dram layout is [128, NBLK*DOUT] (partition p = token b*128+p)
    out = np.concatenate(
        [
            np.asarray(res.results[c]["out"], np.float32)
            .reshape(128, NBLK, DOUT)
            .transpose(1, 0, 2)
            .reshape(NTOK, DOUT)
            for c in range(NCORES)
        ],
        axis=0,
    )
    if _trace:
        _CACHE["last_res"] = res
    return out.reshape(B, S, DOUT).astype(np.float32)
